# revision 1
# baseline (speedup 1.0000x reference)
"""GAT-style message passing (AgentCommunicationModule) on 8 trn2 NeuronCores.

Strategy (per sharding hint): shard destination rows i across 8 cores
(512 rows each); replicate x^T / W / a; each core computes its own
row-block softmax + aggregation; no collectives.

Per-core device algorithm, chunked over j (32 chunks of 128 sources):
  Wx        via PE matmul  (lhsT = x^T chunk, rhs = W^T)        -> [j, 128] f16
  alpha_j   same matmul, extra G columns (G = W^T @ Ablockdiag) -> [j, 8] f32
  P[j,i,h]  = exp(lrelu(alpha_i[i,h] + alpha_j[j,h])) * adjT[j,i], computed via
    - DVE path (most chunks): v,u=0.2v on DVE tensor_scalar (4x fp16),
      max on DVE TT (2x), exp on ACT [128,1024] halves, mask TT (2x)
    - ACT path (KB_CHUNKS odd chunks, balances engines): exp(lrelu(v)) =
      max(exp(v), exp(0.2v)) via two bias-fused ACT Exp ops + DVE max/mask
  numer/den accumulate on PE: lhsT=[Wx_h | ones] (M=33), rhs=P_h -> PSUM [33, 512]
Epilogue: PE-transpose [33,512]->[128,33] per 128-row block, divide, +x,
LayerNorm (bn_stats), gamma/beta on GPSIMD, DMA out.
"""

import sys

for _p in ("/opt/trn_rl_repo",):
    if _p not in sys.path:
        sys.path.insert(0, _p)

from contextlib import ExitStack

import numpy as np

import concourse.bass as bass
import concourse.tile as tile
from concourse import mybir
from concourse.alu_op_type import AluOpType
from concourse.bass_utils import run_bass_kernel_spmd

F32 = mybir.dt.float32
F16 = mybir.dt.float16
U8 = mybir.dt.uint8

N = 4096
F = 128
H = 4
D = 32
NCORES = 8
RB = N // NCORES  # 512 destination rows per core
NCH = N // 128  # 32 source chunks
SLOPE = 0.2
EPS = 1e-5

Exp = mybir.ActivationFunctionType.Exp
Sqrt = mybir.ActivationFunctionType.Sqrt
Ln = mybir.ActivationFunctionType.Ln

USE_CUSTOM_DVE = False
_CUSTOM_OP = None
KB_CHUNKS = 16


def _register_custom_op():
    """Fused DVE op: out = max(v, SLOPE*v) + (adjT-1)*BIG, v = in0 + aj[p]."""
    global _CUSTOM_OP
    if _CUSTOM_OP is not None:
        return _CUSTOM_OP
    import concourse.dve_ops as dve_ops_mod
    from concourse.dve_ops import DveOp
    from concourse.dve_spec import C0, C1, C2, One, Spec, Src0, Src1, lower, maxx
    from concourse.dve_table_gen import dve_ver_for
    from concourse.dve_uop import DveOpSpec

    name = "LRELU_MASK_ANT"
    _v = Src0 + C0
    body = maxx(_v, _v * C1) + (Src1 - One) * C2

    def _ref(in0, in1, s0, s1, imm2):
        v = in0.astype(np.float32) + s0
        return (np.maximum(v, v * s1) + (in1.astype(np.float32) - 1.0) * imm2).astype(
            np.float32
        )

    spec = Spec(body=body, reference=_ref)
    shas = {}
    for ver in ("v3", "v4"):
        s = DveOpSpec(name=name, opcode=None, uops=lower(spec, ver=ver), rd1_en=True)
        shas[ver] = s.sha(ver)
    op = DveOp(name, spec, subdim=False, uops_sha=shas)
    if all(o.name != name for o in dve_ops_mod.OPS):
        dve_ops_mod.OPS.append(op)
        dve_ops_mod._SUB_OPCODE_FOR_NAME[name] = (
            dve_ops_mod._CUSTOM_DVE_ROW_BASE + len(dve_ops_mod.OPS) - 1
        )
        dve_ops_mod.CUSTOM_DVE_SPECS[name] = spec
    _CUSTOM_OP = op
    return op


def _legalize_waits(nc):
    """This walrus build caps sync waits at 1/instruction (2 for
    EventSemaphore). Tile's assigner can emit more; split the excess into
    standalone EventSemaphore waits queued just before the instruction."""
    k = 0
    for f in nc.m.functions:
        for blk in f.blocks:
            out = []
            changed = False
            for ins in blk.instructions:
                si = ins.sync_info
                n = len(si.on_wait) if si else 0
                cap = 2 if isinstance(ins, mybir.InstEventSemaphore) else 1
                if n > cap:
                    waits = list(si.on_wait)
                    keep, extra = waits[-cap:], waits[:-cap]
                    for i in range(0, len(extra), 2):
                        ev = mybir.InstEventSemaphore(
                            name=f"{ins.name}-exw{k}",
                            ins=[],
                            outs=[],
                            engine=ins.engine,
                            sync_info=mybir.SyncInfo(
                                on_wait=extra[i : i + 2], on_update=[]
                            ),
                        )
                        k += 1
                        out.append(ev)
                        changed = True
                    ins.sync_info = mybir.SyncInfo(
                        on_wait=keep, on_update=list(si.on_update)
                    )
                out.append(ins)
            if changed:
                blk.instructions = out
    return nc


def build_nc(reps=1, apply_gb=True):
    nc = bass.Bass(num_swdge_queues=4)
    NCON = 1164
    KB = KB_CHUNKS  # chunks routed to the ACT exp-factorized path
    xT = nc.declare_dram_parameter("xT", [F, N], F32, isOutput=False)
    consts = nc.declare_dram_parameter("consts", [F, NCON], F32, isOutput=False)
    adjT = nc.declare_dram_parameter("adjT", [N, RB], U8, isOutput=False)
    x_own = nc.declare_dram_parameter("x_own", [RB, F], F32, isOutput=False)
    yout = nc.declare_dram_parameter("y", [RB, F], F32, isOutput=True)

    add = AluOpType.add
    mult = AluOpType.mult
    amax = AluOpType.max
    sub = AluOpType.subtract

    cop = _register_custom_op() if USE_CUSTOM_DVE else None

    with tile.TileContext(nc) as tc, ExitStack() as ctx:
        sing = ctx.enter_context(tc.tile_pool(name="sing", bufs=1))
        small = ctx.enter_context(tc.tile_pool(name="small", bufs=4))
        xt_p = ctx.enter_context(tc.tile_pool(name="xt", bufs=4))
        adj_p = ctx.enter_context(tc.tile_pool(name="adj", bufs=4))
        u_p = ctx.enter_context(tc.tile_pool(name="u", bufs=12))
        p_p = ctx.enter_context(tc.tile_pool(name="p", bufs=4))
        e_p = ctx.enter_context(tc.tile_pool(name="e", bufs=4))
        pm_p = ctx.enter_context(tc.tile_pool(name="pm", bufs=4))
        w_p = ctx.enter_context(tc.tile_pool(name="w", bufs=4))
        epi_p = ctx.enter_context(tc.tile_pool(name="epi", bufs=4))
        ps_agg = ctx.enter_context(tc.tile_pool(name="ps_agg", bufs=1, space="PSUM"))
        ps_s = ctx.enter_context(tc.tile_pool(name="ps_s", bufs=3, space="PSUM"))
        ps_aiT = ctx.enter_context(tc.tile_pool(name="ps_aiT", bufs=1, space="PSUM"))
        dram_p = ctx.enter_context(tc.tile_pool(name="dram_p", bufs=1, space="DRAM"))

        # ---- load packed constants in ONE DMA ----
        con_sb = sing.tile([F, NCON], F32, tag="con_sb")
        nc.sync.dma_start(con_sb[:, 0:268], consts[:, 0:268])
        nc.sync.dma_start(con_sb[:, 268:NCON], consts[:, 268:NCON])
        W_sb = con_sb[:, 0:128]
        WT_sb = con_sb[:, 128:256]
        AB_sb = con_sb[:, 256:268]
        eye_sb = con_sb[:, 268:396]
        xoT_sb = con_sb[:, 396:908]
        gamma_row = con_sb[0:1, 908:1036]
        beta_row = con_sb[0:1, 1036:1164]
        eps_sb = sing.tile([F, 1], F32, tag="eps_sb")
        nc.vector.memset(eps_sb, EPS)
        warm = sing.tile([1, 1], F32, tag="warm")
        nc.scalar.activation(warm, eps_sb[0:1, 0:1], Exp)
        if apply_gb:
            gamma_rep = sing.tile([F, F], F32, tag="gamma_rep")
            beta_rep = sing.tile([F, F], F32, tag="beta_rep")
            nc.gpsimd.dma_start(
                out=gamma_rep,
                in_=bass.AP(
                    tensor=consts[0:1, 908:1036].tensor, offset=908,
                    ap=[[0, F], [1, F]],
                ),
            )
            nc.gpsimd.dma_start(
                out=beta_rep,
                in_=bass.AP(
                    tensor=consts[0:1, 1036:1164].tensor, offset=1036,
                    ap=[[0, F], [1, F]],
                ),
            )

        # ---- G = W^T @ AB  (columns 0:4 -> alpha_i weights, 4:8 -> alpha_j) ----
        psum_g = ps_s.tile([F, 3 * H], F32, tag="ps", padded_shape=[F, 512])
        nc.tensor.matmul(psum_g, lhsT=W_sb, rhs=AB_sb, start=True, stop=True)
        G_sb = sing.tile([F, 3 * H], F32, tag="G_sb")
        nc.vector.tensor_copy(G_sb, psum_g)
        # pack [G_j | 0.2*G_j] right after WT so each chunk needs ONE matmul
        GJ_sb = sing.tile([F, 2 * H], F32, tag="GJ_sb")
        nc.vector.tensor_copy(GJ_sb, G_sb[:, H : 3 * H])

        # ---- alpha_i for own rows: one matmul  [4,512] = G_i.T @ xoT ----
        psum_aiT = ps_aiT.tile([H, RB], F32, tag="psum_aiT")
        nc.tensor.matmul(psum_aiT, lhsT=G_sb[:, 0:H], rhs=xoT_sb, start=True, stop=True)
        aiT_sb = sing.tile([H, RB], F32, tag="aiT_sb")
        nc.vector.tensor_copy(aiT_sb, psum_aiT)
        aiT_dram = dram_p.tile([H, RB], F16, tag="aiT_dram")
        nc.gpsimd.dma_start(aiT_dram, aiT_sb)
        airep_all = sing.tile([F, H * RB], F16, tag="airep_all")
        for h in range(0, H, 2):
            nc.gpsimd.dma_start(
                out=airep_all[:, h * RB : (h + 2) * RB],
                in_=bass.AP(
                    tensor=aiT_dram.tensor,
                    offset=aiT_dram.offset + h * RB,
                    ap=[[0, F], [RB, 2], [1, RB]],
                ),
            )
        airep = [airep_all[:, h * RB : (h + 1) * RB] for h in range(H)]

        # ---- aggregation accumulators (live across all chunks) ----
        for rep in range(reps):
          agg = [ps_agg.tile([D + 1, RB], F32, tag=f"agg{h}", name=f"agg{h}_{rep}") for h in range(H)]

          # ---- main loop over source chunks ----
          for c in range(NCH):
              xTc = xt_p.tile([F, F], F32, tag="xTc")
              nc.sync.dma_start(xTc, xT[:, c * F : (c + 1) * F])
              T_c = adj_p.tile([F, RB], F16, tag="T_c")
              nc.gpsimd.dma_start(out=T_c, in_=adjT[c * F : (c + 1) * F, :])

              psum_w = ps_s.tile(
                  [F, F + 2 * H], F32, tag="ps", name="psum_w", padded_shape=[F, 512]
              )
              nc.tensor.matmul(
                  psum_w[:, 0:F], lhsT=xTc, rhs=WT_sb, start=True, stop=True
              )
              nc.tensor.matmul(
                  psum_w[:, F : F + 2 * H], lhsT=xTc, rhs=G_sb[:, H : 3 * H],
                  start=True, stop=True,
              )
              wx = w_p.tile([F, (D + 1) * H], F16, tag="wx")
              nc.vector.memset(wx, 1.0)
              is_act_chunk = (c % 2 == 1) and (c >= NCH - 2 * min(KB, 16) + 1)
              if KB > 16:
                  is_act_chunk = is_act_chunk or (c < 2 * (KB - 16))
              if c == NCH - 2:
                  is_act_chunk = True
              if c == NCH - 1:
                  is_act_chunk = False
              wx_r = wx.rearrange("p (h q) -> p h q", h=H)[:, :, 0:D]
              ps_r = psum_w[:, 0:F].rearrange("p (h d) -> p h d", h=H)
              ajc = small.tile([F, 2 * H], F32, tag="ajc", bufs=6)
              nc.vector.tensor_copy(wx_r, ps_r)
              nc.vector.tensor_copy(ajc, psum_w[:, F : F + 2 * H])

              PM_c = pm_p.tile([F, H * RB], F16, tag="PM_c")
              if is_act_chunk:
                  # ACT path: exp(lrelu(v)) = max(exp(v), exp(0.2 v))
                  mx_all = u_p.tile([F, H * RB], F16, tag="mx_all", bufs=3)
                  e1_all = u_p.tile([F, H * RB], F16, tag="e1_all", bufs=3)
                  e2_all = u_p.tile([F, H * RB], F16, tag="e2_all", bufs=3)
                  for h in range(H):
                      nc.scalar.activation(
                          e1_all[:, h * RB : (h + 1) * RB], airep[h], Exp,
                          bias=ajc[:, h : h + 1], scale=1.0,
                      )
                      nc.scalar.activation(
                          e2_all[:, h * RB : (h + 1) * RB], airep[h], Exp,
                          bias=ajc[:, H + h : H + h + 1], scale=SLOPE,
                      )
                  nc.vector.tensor_tensor(
                      out=mx_all, in0=e1_all, in1=e2_all, op=amax
                  )
                  t_bc = bass.AP(
                      tensor=T_c.tensor,
                      offset=T_c.offset,
                      ap=[T_c.ap[0], [0, H], [1, RB]],
                  )
                  nc.vector.tensor_tensor(out=PM_c, in0=mx_all, in1=t_bc, op=mult)
                  for h in range(H):
                      nc.tensor.matmul(
                          agg[h],
                          lhsT=wx[:, h * (D + 1) : (h + 1) * (D + 1)],
                          rhs=PM_c[:, h * RB : (h + 1) * RB],
                          start=(c == 0),
                          stop=(c == NCH - 1),
                      )
                  continue
              u_all = u_p.tile([F, H * RB], F16, tag="u_all", bufs=3)
              v_all = u_p.tile([F, H * RB], F16, tag="v_all", bufs=3)
              for h in range(H):
                  nc.vector.tensor_scalar(
                      out=v_all[:, h * RB : (h + 1) * RB], in0=airep[h],
                      scalar1=ajc[:, h : h + 1], scalar2=None, op0=add,
                  )
              nc.vector.tensor_scalar(
                  out=u_all, in0=v_all, scalar1=SLOPE, scalar2=None, op0=mult
              )
              P_c = p_p.tile([F, H * RB], F16, tag="P_c")
              # max over both halves separately so exp/mask can start earlier
              HH = H * RB // 2
              E_c = e_p.tile([F, H * RB], F16, tag="E_c")
              for half in range(2):
                  sl = slice(half * HH, (half + 1) * HH)
                  nc.vector.tensor_tensor(
                      out=P_c[:, sl], in0=v_all[:, sl], in1=u_all[:, sl], op=amax
                  )
                  nc.scalar.activation(E_c[:, sl], P_c[:, sl], Exp)
                  t_bc = bass.AP(
                      tensor=T_c.tensor,
                      offset=T_c.offset,
                      ap=[T_c.ap[0], [0, H // 2], [1, RB]],
                  )
                  nc.vector.tensor_tensor(
                      out=PM_c[:, sl], in0=E_c[:, sl], in1=t_bc, op=mult
                  )

              for h in range(H):
                  nc.tensor.matmul(
                      agg[h],
                      lhsT=wx[:, h * (D + 1) : (h + 1) * (D + 1)],
                      rhs=PM_c[:, h * RB : (h + 1) * RB],
                      start=(c == 0),
                      stop=(c == NCH - 1),
                  )

          # ---- epilogue ----
          aggsb = []
          for h in range(H):
              t = sing.tile([D + 1, RB], F32, tag=f"aggsb{h}", name=f"aggsb{h}")
              nc.scalar.copy(t[:, 0 : RB // 2], agg[h][:, 0 : RB // 2])
              nc.scalar.copy(t[:, RB // 2 : RB], agg[h][:, RB // 2 : RB])
              aggsb.append(t)

          for s in range(RB // F):
              psum_t = ps_s.tile([F, (D + 1) * H], F32, tag="ps", name="psum_t", padded_shape=[F, 512])
              for h in range(H):
                  nc.tensor.transpose(
                      psum_t[:, h * (D + 1) : (h + 1) * (D + 1)],
                      aggsb[h][:, s * F : (s + 1) * F],
                      eye_sb[0 : D + 1, 0 : D + 1],
                  )
              recips = epi_p.tile([F, H], F32, tag="recips")
              for h in range(H):
                  nc.vector.reciprocal(
                      recips[:, h : h + 1],
                      psum_t[:, h * (D + 1) + D : h * (D + 1) + D + 1],
                  )
              y_s = epi_p.tile([F, F], F32, tag="y_s")
              for h in range(H):
                  nc.vector.tensor_scalar(
                      out=y_s[:, h * D : (h + 1) * D],
                      in0=psum_t[:, h * (D + 1) : h * (D + 1) + D],
                      scalar1=recips[:, h : h + 1],
                      scalar2=None,
                      op0=mult,
                  )
              x_s = epi_p.tile([F, F], F32, tag="x_s")
              nc.sync.dma_start(x_s, x_own[s * F : (s + 1) * F, :])
              nc.gpsimd.tensor_tensor(out=y_s, in0=y_s, in1=x_s, op=add)
              stats = epi_p.tile([F, 6], F32, tag="stats")
              nc.vector.bn_stats(out=stats, in_=y_s)
              mv = epi_p.tile([F, 2], F32, tag="mv")
              nc.vector.bn_aggr(out=mv, in_=stats)
              lnv = epi_p.tile([F, 1], F32, tag="lnv")
              nc.scalar.activation(lnv, mv[:, 1:2], Ln, bias=eps_sb, scale=1.0)
              rstd = epi_p.tile([F, 1], F32, tag="rstd")
              nc.scalar.activation(rstd, lnv, Exp, scale=-0.5)
              nc.vector.tensor_scalar(
                  out=y_s, in0=y_s, scalar1=mv[:, 0:1], scalar2=rstd,
                  op0=sub, op1=mult,
              )
              if apply_gb:
                  nc.gpsimd.tensor_tensor(out=y_s, in0=y_s, in1=gamma_rep, op=mult)
                  nc.gpsimd.tensor_tensor(out=y_s, in0=y_s, in1=beta_rep, op=add)
              nc.sync.dma_start(yout[s * F : (s + 1) * F, :], y_s)

    return _legalize_waits(nc)


_NC_CACHE = {}


def _get_nc(reps=1, apply_gb=True):
    key = (reps, apply_gb)
    if key not in _NC_CACHE:
        _NC_CACHE[key] = build_nc(reps, apply_gb)
    return _NC_CACHE[key]


def _host_inputs(x, adjacency, W, a, ln_gamma, ln_beta):
    x = np.asarray(x, dtype=np.float32)
    adjacency = np.asarray(adjacency)
    W = np.asarray(W, dtype=np.float32)
    a = np.asarray(a, dtype=np.float32)
    xT = np.ascontiguousarray(x.T)
    AB = np.zeros((F, 3 * H), dtype=np.float32)
    for h in range(H):
        AB[h * D : (h + 1) * D, h] = a[h, :D]
        AB[h * D : (h + 1) * D, H + h] = a[h, D:]
        AB[h * D : (h + 1) * D, 2 * H + h] = 0.2 * a[h, D:]
    adjT_full = np.ascontiguousarray(adjacency.T).view(np.uint8)
    maps = []
    for c in range(NCORES):
        sl = slice(c * RB, (c + 1) * RB)
        con = np.zeros((F, 1164), dtype=np.float32)
        con[:, 0:128] = W
        con[:, 128:256] = W.T
        con[:, 256:268] = AB
        con[:, 268:396] = np.eye(F, dtype=np.float32)
        con[:, 396:908] = xT[:, sl]
        con[0, 908:1036] = np.asarray(ln_gamma, np.float32)
        con[0, 1036:1164] = np.asarray(ln_beta, np.float32)
        maps.append(
            dict(
                xT=xT,
                consts=con,
                adjT=np.ascontiguousarray(adjT_full[:, sl]),
                x_own=np.ascontiguousarray(x[sl]),
            )
        )
    return maps


def run_on_cores(inputs, **run_kwargs):
    g = np.asarray(inputs["ln_gamma"], np.float32)
    b = np.asarray(inputs["ln_beta"], np.float32)
    apply_gb = not (np.all(g == 1.0) and np.all(b == 0.0))
    nc = _get_nc(apply_gb=apply_gb)
    maps = _host_inputs(**inputs)
    return run_bass_kernel_spmd(nc, maps, list(range(NCORES)), **run_kwargs)


def kernel(**inputs) -> np.ndarray:
    res = run_on_cores(inputs)
    return np.concatenate(
        [res.results[i]["y"] for i in range(NCORES)], axis=0
    ).astype(np.float32)


if __name__ == "__main__":
    rng = np.random.default_rng(0)
    x = rng.standard_normal((N, F), dtype=np.float32)
    adj = rng.integers(0, 2, size=(N, N)).astype(bool)
    W = rng.standard_normal((F, F), dtype=np.float32) * 0.088
    a = rng.standard_normal((H, 2 * D), dtype=np.float32) * 0.17
    y = kernel(
        x=x, adjacency=adj, W=W, a=a,
        ln_gamma=np.ones(F, np.float32), ln_beta=np.zeros(F, np.float32),
    )
    print(y.shape, y.dtype)



# revision 12
# speedup vs baseline: 32.5603x; 32.5603x over previous
"""GAT-style message passing (AgentCommunicationModule) on 8 trn2 NeuronCores.

Strategy (per sharding hint): shard destination rows i across 8 cores
(512 rows each); replicate x^T / W / a; each core computes its own
row-block softmax + aggregation; no collectives.

Math: P[j,i,h] = exp(lrelu(ai[i,h]+aj[j,h]))*adj[j,i]
             = max(exp(ai)exp(aj), exp(.2ai)exp(.2aj))*adj  (exact)
so with A1=exp(ai), A2=exp(.2ai) replicated once per core and
B1=exp(aj), B2=exp(.2aj) per-partition scalars per chunk, P needs only
TensorScalar (4x-mode) + max + mask per chunk ("TS chunks").  A share of
chunks instead computes e1/e2 = Exp(ai+aj) / Exp(.2(ai+aj)) on the ACT
engine ("ACT chunks").  The max/mask TensorTensor ops are assigned to
DVE or Pool (gpsimd) by a greedy build-time load balancer.  Adjacency is
host-converted to f16 so its DMA needs no cast and issues from SP
instead of Pool.

numer/den accumulate on PE: lhsT=[Wx_h | ones] (M=33), rhs=P_h -> PSUM [33, 512]
Epilogue: PE-transpose [33,512]->[128,33] per 128-row block, divide, +x,
LayerNorm (bn_stats), gamma/beta on GPSIMD, DMA out.
"""

import sys

for _p in ("/opt/trn_rl_repo",):
    if _p not in sys.path:
        sys.path.insert(0, _p)

from contextlib import ExitStack

import numpy as np

import concourse.bass as bass
import concourse.tile as tile
from concourse import mybir
from concourse.alu_op_type import AluOpType
from concourse.bass_utils import run_bass_kernel_spmd

F32 = mybir.dt.float32
F16 = mybir.dt.float16

N = 4096
F = 128
H = 4
D = 32
NCORES = 8
RB = N // NCORES  # 512 destination rows per core
NCH = N // 128  # 32 source chunks
SLOPE = 0.2
EPS = 1e-5

Exp = mybir.ActivationFunctionType.Exp
Ln = mybir.ActivationFunctionType.Ln

# ---- tunables -------------------------------------------------------------
# which chunks use the ACT-engine exp path (others use the TS path)
N_ACT_CHUNKS = 13
# cost table (ns) for the greedy DVE/Pool balancer, calibrated on the
# TimelineSim cost model
C_TT_V = 664  # 1024-col f16 TT on DVE
C_MAX_P = 1460  # 1024-col f16 max on Pool
C_MUL_P = 1993  # 1024-col f16 mult on Pool
POOL_BASE = 12000  # epilogue + one-time bcast DMA issue on Pool
DVE_BASE = 20000  # copies/memsets/recips etc on DVE
ACT_BASE = 6000
N_WX_BUFS = 6


def _act_chunk_set(n_act=N_ACT_CHUNKS):
    if n_act <= 0:
        return set()
    pos = np.linspace(0, NCH - 1, n_act)
    return set(int(round(p)) for p in pos)


def _legalize_waits(nc):
    """This walrus build caps sync waits at 1/instruction (2 for
    EventSemaphore). Tile's assigner can emit more; split the excess into
    standalone EventSemaphore waits queued just before the instruction."""
    k = 0
    for f in nc.m.functions:
        for blk in f.blocks:
            out = []
            changed = False
            for ins in blk.instructions:
                si = ins.sync_info
                n = len(si.on_wait) if si else 0
                cap = 2 if isinstance(ins, mybir.InstEventSemaphore) else 1
                if n > cap:
                    waits = list(si.on_wait)
                    keep, extra = waits[-cap:], waits[:-cap]
                    for i in range(0, len(extra), 2):
                        ev = mybir.InstEventSemaphore(
                            name=f"{ins.name}-exw{k}",
                            ins=[],
                            outs=[],
                            engine=ins.engine,
                            sync_info=mybir.SyncInfo(
                                on_wait=extra[i : i + 2], on_update=[]
                            ),
                        )
                        k += 1
                        out.append(ev)
                        changed = True
                    ins.sync_info = mybir.SyncInfo(
                        on_wait=keep, on_update=list(si.on_update)
                    )
                out.append(ins)
            if changed:
                blk.instructions = out
    return nc


class _Sched:
    """Greedy engine picker balancing estimated busy-ns."""

    def __init__(self):
        self.busy = {"V": float(DVE_BASE), "P": float(POOL_BASE), "A": float(ACT_BASE)}

    def pick(self, cands):
        eng, cost = min(cands, key=lambda ec: self.busy[ec[0]] + ec[1])
        self.busy[eng] += cost
        return eng


def build_nc(reps=1, apply_gb=True, n_act=None):
    nc = bass.Bass(num_swdge_queues=4)
    NCON = 1164
    xT = nc.declare_dram_parameter("xT", [F, N], F32, isOutput=False)
    consts = nc.declare_dram_parameter("consts", [F, NCON], F32, isOutput=False)
    adjT = nc.declare_dram_parameter("adjT", [N, RB], F16, isOutput=False)
    x_own = nc.declare_dram_parameter("x_own", [RB, F], F32, isOutput=False)
    yout = nc.declare_dram_parameter("y", [RB, F], F32, isOutput=True)

    add = AluOpType.add
    mult = AluOpType.mult
    amax = AluOpType.max
    sub = AluOpType.subtract

    act_set = _act_chunk_set(N_ACT_CHUNKS if n_act is None else n_act)

    with tile.TileContext(nc) as tc, ExitStack() as ctx:
        sing = ctx.enter_context(tc.tile_pool(name="sing", bufs=1))
        small = ctx.enter_context(tc.tile_pool(name="small", bufs=4))
        xt_p = ctx.enter_context(tc.tile_pool(name="xt", bufs=4))
        adj_p = ctx.enter_context(tc.tile_pool(name="adj", bufs=4))
        u_p = ctx.enter_context(tc.tile_pool(name="u", bufs=8))
        p_p = ctx.enter_context(tc.tile_pool(name="p", bufs=3))
        pm_p = ctx.enter_context(tc.tile_pool(name="pm", bufs=4))
        w_p = ctx.enter_context(tc.tile_pool(name="w", bufs=4))
        epi_p = ctx.enter_context(tc.tile_pool(name="epi", bufs=4))
        ps_agg = ctx.enter_context(tc.tile_pool(name="ps_agg", bufs=1, space="PSUM"))
        ps_s = ctx.enter_context(tc.tile_pool(name="ps_s", bufs=4, space="PSUM"))
        dram_p = ctx.enter_context(tc.tile_pool(name="dram_p", bufs=1, space="DRAM"))

        # ---- load packed constants ----
        con_sb = sing.tile([F, NCON], F32, tag="con_sb")
        nc.sync.dma_start(con_sb[:, 0:268], consts[:, 0:268])
        nc.sync.dma_start(con_sb[:, 268:NCON], consts[:, 268:NCON])
        W_sb = con_sb[:, 0:128]
        WT_sb = con_sb[:, 128:256]
        AB_sb = con_sb[:, 256:268]
        eye_sb = con_sb[:, 268:396]
        xoT_sb = con_sb[:, 396:908]
        eps_sb = sing.tile([F, 1], F32, tag="eps_sb")
        nc.vector.memset(eps_sb, EPS)
        warm = sing.tile([1, 1], F32, tag="warm")
        nc.scalar.activation(warm, eps_sb[0:1, 0:1], Exp)

        # ---- G = W^T @ AB  (cols 0:H -> ai weights, H:2H -> aj, 2H:3H -> .2aj) ----
        psum_g = ps_s.tile([F, 3 * H], F32, tag="ps", padded_shape=[F, 512])
        nc.tensor.matmul(psum_g, lhsT=W_sb, rhs=AB_sb, start=True, stop=True)
        G_sb = sing.tile([F, 3 * H], F32, tag="G_sb")
        nc.vector.tensor_copy(G_sb, psum_g)

        # ---- alpha_i for own rows: [H,512] = G_i.T @ xoT ----
        psum_aiT = ps_s.tile([H, RB], F32, tag="ps", name="psum_aiT", padded_shape=[F, 512])
        nc.tensor.matmul(psum_aiT, lhsT=G_sb[:, 0:H], rhs=xoT_sb, start=True, stop=True)
        aiT_sb = sing.tile([H, RB], F32, tag="aiT_sb")
        nc.vector.tensor_copy(aiT_sb, psum_aiT)
        # raw ai (f16) + exp(ai), exp(.2 ai) — staged via DRAM for the
        # partition-broadcast loads
        aiT_dram = dram_p.tile([H, 3 * RB], F16, tag="aiT_dram")
        nc.gpsimd.dma_start(aiT_dram[:, 0:RB], aiT_sb)
        a1T_sb = sing.tile([H, RB], F16, tag="a1T_sb")
        a2T_sb = sing.tile([H, RB], F16, tag="a2T_sb")
        nc.scalar.activation(a1T_sb, aiT_sb, Exp, scale=1.0)
        nc.scalar.activation(a2T_sb, aiT_sb, Exp, scale=SLOPE)
        nc.gpsimd.dma_start(aiT_dram[:, RB : 2 * RB], a1T_sb)
        nc.gpsimd.dma_start(aiT_dram[:, 2 * RB : 3 * RB], a2T_sb)

        def bcast_all(dst, col0):
            # replicate [H, RB] (cols col0:col0+RB of aiT_dram) across the
            # 128 partitions -> dst [F, H*RB], 2 heads per DMA
            for h in range(0, H, 2):
                nc.gpsimd.dma_start(
                    out=dst[:, h * RB : (h + 2) * RB],
                    in_=bass.AP(
                        tensor=aiT_dram.tensor,
                        offset=aiT_dram.offset + h * 3 * RB + col0,
                        ap=[[0, F], [3 * RB, 2], [1, RB]],
                    ),
                )

        airep_all = sing.tile([F, H * RB], F16, tag="airep_all")
        a1rep_all = sing.tile([F, H * RB], F16, tag="a1rep_all")
        a2rep_all = sing.tile([F, H * RB], F16, tag="a2rep_all")
        bcast_all(airep_all, 0)
        bcast_all(a1rep_all, RB)
        bcast_all(a2rep_all, 2 * RB)
        if apply_gb:
            gamma_rep = sing.tile([F, F], F32, tag="gamma_rep")
            beta_rep = sing.tile([F, F], F32, tag="beta_rep")
            nc.gpsimd.dma_start(
                out=gamma_rep,
                in_=bass.AP(
                    tensor=consts[0:1, 908:1036].tensor, offset=908,
                    ap=[[0, F], [1, F]],
                ),
            )
            nc.gpsimd.dma_start(
                out=beta_rep,
                in_=bass.AP(
                    tensor=consts[0:1, 1036:1164].tensor, offset=1036,
                    ap=[[0, F], [1, F]],
                ),
            )

        airep = [airep_all[:, h * RB : (h + 1) * RB] for h in range(H)]
        a1rep = [a1rep_all[:, h * RB : (h + 1) * RB] for h in range(H)]
        a2rep = [a2rep_all[:, h * RB : (h + 1) * RB] for h in range(H)]

        # persistent wx buffers: ones column written once, Wx part rewritten
        # per chunk (round-robin over 4)
        wx_bufs = []
        for i in range(N_WX_BUFS):
            t = sing.tile([F, (D + 1) * H], F16, tag=f"wx{i}", name=f"wx{i}")
            nc.vector.memset(t, 1.0)
            wx_bufs.append(t)

        HH = H * RB // 2  # 1024-col halves for DVE/Pool balancing

        for rep in range(reps):
          sched = _Sched()
          agg = [
              ps_agg.tile([D + 1, RB], F32, tag=f"agg{h}", name=f"agg{h}_{rep}")
              for h in range(H)
          ]

          # ---- main loop over source chunks, software-pipelined in 4
          # stages with per-stage skews so each engine's in-order queue
          # interleaves stages of adjacent chunks and PE's agg matmuls
          # never block the next chunk's Wx matmul.
          state = {}  # chunk -> dict

          def emit_load(c):
              xTc = xt_p.tile([F, F], F32, tag="xTc", name=f"xTc_{rep}_{c}")
              nc.sync.dma_start(xTc, xT[:, c * F : (c + 1) * F])
              T_c = adj_p.tile([F, RB], F16, tag="T_c", name=f"T_c_{rep}_{c}")
              nc.sync.dma_start(T_c, adjT[c * F : (c + 1) * F, :])

              psum_w = ps_s.tile(
                  [F, F + 2 * H], F32, tag="ps", name=f"psum_w_{rep}_{c}",
                  padded_shape=[F, 512],
              )
              nc.tensor.matmul(
                  psum_w[:, 0:F], lhsT=xTc, rhs=WT_sb, start=True, stop=True
              )
              nc.tensor.matmul(
                  psum_w[:, F : F + 2 * H], lhsT=xTc, rhs=G_sb[:, H : 3 * H],
                  start=True, stop=True,
              )
              state[c] = dict(psum_w=psum_w, T_c=T_c)

          def emit_gen(c):
              st = state[c]
              psum_w = st.pop("psum_w")
              wx = wx_bufs[c % N_WX_BUFS]
              wx_r = wx.rearrange("p (h q) -> p h q", h=H)[:, :, 0:D]
              ps_r = psum_w[:, 0:F].rearrange("p (h d) -> p h d", h=H)
              ajc = small.tile([F, 2 * H], F32, tag="ajc", bufs=6, name=f"ajc_{rep}_{c}")
              weng = sched.pick([("V", 400), ("A", 800)])
              (nc.vector.tensor_copy if weng == "V" else nc.scalar.copy)(wx_r, ps_r)
              nc.vector.tensor_copy(ajc, psum_w[:, F : F + 2 * H])

              g1 = u_p.tile([F, H * RB], F16, tag="g1", bufs=4, name=f"g1_{rep}_{c}")
              g2 = u_p.tile([F, H * RB], F16, tag="g2", bufs=4, name=f"g2_{rep}_{c}")
              if c in act_set:
                  # ACT path: e1 = exp(ai+aj), e2 = exp(.2(ai+aj))
                  for h in range(H):
                      nc.scalar.activation(
                          g1[:, h * RB : (h + 1) * RB], airep[h], Exp,
                          bias=ajc[:, h : h + 1], scale=1.0,
                      )
                      nc.scalar.activation(
                          g2[:, h * RB : (h + 1) * RB], airep[h], Exp,
                          bias=ajc[:, H + h : H + h + 1], scale=SLOPE,
                      )
              else:
                  # TS path: t1 = exp(ai)*exp(aj), t2 = exp(.2ai)*exp(.2aj)
                  bexp = small.tile(
                      [F, 2 * H], F32, tag="bexp", bufs=6, name=f"bexp_{rep}_{c}"
                  )
                  nc.scalar.activation(bexp, ajc, Exp)
                  for h in range(H):
                      nc.vector.tensor_scalar(
                          out=g1[:, h * RB : (h + 1) * RB], in0=a1rep[h],
                          scalar1=bexp[:, h : h + 1], scalar2=None, op0=mult,
                      )
                      nc.vector.tensor_scalar(
                          out=g2[:, h * RB : (h + 1) * RB], in0=a2rep[h],
                          scalar1=bexp[:, H + h : H + h + 1], scalar2=None, op0=mult,
                      )
              st.update(g1=g1, g2=g2, wx=wx)

          def emit_mask(c):
              st = state[c]
              g1, g2, T_c = st.pop("g1"), st.pop("g2"), st.pop("T_c")
              E_c = p_p.tile([F, H * RB], F16, tag="E_c", name=f"E_c_{rep}_{c}")
              PM_c = pm_p.tile([F, H * RB], F16, tag="PM_c", name=f"PM_c_{rep}_{c}")
              halves = [slice(0, HH), slice(HH, 2 * HH)]
              for sl in halves:
                  # Pool's ISA has no TT-max; DVE only
                  nc.vector.tensor_tensor(
                      out=E_c[:, sl], in0=g1[:, sl], in1=g2[:, sl], op=amax
                  )
                  sched.busy["V"] += C_TT_V
              t_bc = bass.AP(
                  tensor=T_c.tensor,
                  offset=T_c.offset,
                  ap=[T_c.ap[0], [0, H // 2], [1, RB]],
              )
              for sl in halves:
                  eng = sched.pick([("V", C_TT_V), ("P", C_MUL_P)])
                  dev = nc.vector if eng == "V" else nc.gpsimd
                  dev.tensor_tensor(out=PM_c[:, sl], in0=E_c[:, sl], in1=t_bc, op=mult)
              st["PM"] = PM_c

          def emit_agg(c):
              st = state.pop(c)
              PM_c, wx = st["PM"], st["wx"]
              for h in range(H):
                  nc.tensor.matmul(
                      agg[h],
                      lhsT=wx[:, h * (D + 1) : (h + 1) * (D + 1)],
                      rhs=PM_c[:, h * RB : (h + 1) * RB],
                      start=(c == 0),
                      stop=(c == NCH - 1),
                  )

          for k in range(NCH + 3):
              if k < NCH:
                  emit_load(k)
              if 1 <= k < NCH + 1:
                  emit_gen(k - 1)
              if 2 <= k < NCH + 2:
                  emit_mask(k - 2)
              if k >= 3:
                  emit_agg(k - 3)

          # ---- epilogue ----
          aggsb = []
          for h in range(H):
              t = sing.tile([D + 1, RB], F32, tag=f"aggsb{h}", name=f"aggsb{h}_{rep}")
              nc.scalar.copy(t[:, 0 : RB // 2], agg[h][:, 0 : RB // 2])
              nc.scalar.copy(t[:, RB // 2 : RB], agg[h][:, RB // 2 : RB])
              aggsb.append(t)

          for s in range(RB // F):
              psum_t = ps_s.tile([F, (D + 1) * H], F32, tag="ps", name=f"psum_t_{rep}_{s}", padded_shape=[F, 512])
              for h in range(H):
                  nc.tensor.transpose(
                      psum_t[:, h * (D + 1) : (h + 1) * (D + 1)],
                      aggsb[h][:, s * F : (s + 1) * F],
                      eye_sb[0 : D + 1, 0 : D + 1],
                  )
              recips = epi_p.tile([F, H], F32, tag="recips")
              for h in range(H):
                  nc.vector.reciprocal(
                      recips[:, h : h + 1],
                      psum_t[:, h * (D + 1) + D : h * (D + 1) + D + 1],
                  )
              y_s = epi_p.tile([F, F], F32, tag="y_s")
              for h in range(H):
                  nc.vector.tensor_scalar(
                      out=y_s[:, h * D : (h + 1) * D],
                      in0=psum_t[:, h * (D + 1) : h * (D + 1) + D],
                      scalar1=recips[:, h : h + 1],
                      scalar2=None,
                      op0=mult,
                  )
              x_s = epi_p.tile([F, F], F32, tag="x_s")
              nc.sync.dma_start(x_s, x_own[s * F : (s + 1) * F, :])
              nc.gpsimd.tensor_tensor(out=y_s, in0=y_s, in1=x_s, op=add)
              stats = epi_p.tile([F, 6], F32, tag="stats")
              nc.vector.bn_stats(out=stats, in_=y_s)
              mv = epi_p.tile([F, 2], F32, tag="mv")
              nc.vector.bn_aggr(out=mv, in_=stats)
              lnv = epi_p.tile([F, 1], F32, tag="lnv")
              nc.scalar.activation(lnv, mv[:, 1:2], Ln, bias=eps_sb, scale=1.0)
              rstd = epi_p.tile([F, 1], F32, tag="rstd")
              nc.scalar.activation(rstd, lnv, Exp, scale=-0.5)
              nc.vector.tensor_scalar(
                  out=y_s, in0=y_s, scalar1=mv[:, 0:1], scalar2=rstd,
                  op0=sub, op1=mult,
              )
              if apply_gb:
                  nc.gpsimd.tensor_tensor(out=y_s, in0=y_s, in1=gamma_rep, op=mult)
                  nc.gpsimd.tensor_tensor(out=y_s, in0=y_s, in1=beta_rep, op=add)
              nc.sync.dma_start(yout[s * F : (s + 1) * F, :], y_s)

    return _legalize_waits(nc)


_NC_CACHE = {}


def _get_nc(reps=1, apply_gb=True):
    key = (reps, apply_gb)
    if key not in _NC_CACHE:
        _NC_CACHE[key] = build_nc(reps, apply_gb)
    return _NC_CACHE[key]


def _host_inputs(x, adjacency, W, a, ln_gamma, ln_beta):
    x = np.asarray(x, dtype=np.float32)
    adjacency = np.asarray(adjacency)
    W = np.asarray(W, dtype=np.float32)
    a = np.asarray(a, dtype=np.float32)
    xT = np.ascontiguousarray(x.T)
    AB = np.zeros((F, 3 * H), dtype=np.float32)
    for h in range(H):
        AB[h * D : (h + 1) * D, h] = a[h, :D]
        AB[h * D : (h + 1) * D, H + h] = a[h, D:]
        AB[h * D : (h + 1) * D, 2 * H + h] = 0.2 * a[h, D:]
    adjT_full = np.ascontiguousarray(adjacency.T).astype(np.float16)
    maps = []
    for c in range(NCORES):
        sl = slice(c * RB, (c + 1) * RB)
        con = np.zeros((F, 1164), dtype=np.float32)
        con[:, 0:128] = W
        con[:, 128:256] = W.T
        con[:, 256:268] = AB
        con[:, 268:396] = np.eye(F, dtype=np.float32)
        con[:, 396:908] = xT[:, sl]
        con[0, 908:1036] = np.asarray(ln_gamma, np.float32)
        con[0, 1036:1164] = np.asarray(ln_beta, np.float32)
        maps.append(
            dict(
                xT=xT,
                consts=con,
                adjT=np.ascontiguousarray(adjT_full[:, sl]),
                x_own=np.ascontiguousarray(x[sl]),
            )
        )
    return maps


def run_on_cores(inputs, **run_kwargs):
    g = np.asarray(inputs["ln_gamma"], np.float32)
    b = np.asarray(inputs["ln_beta"], np.float32)
    apply_gb = not (np.all(g == 1.0) and np.all(b == 0.0))
    nc = _get_nc(apply_gb=apply_gb)
    maps = _host_inputs(**inputs)
    return run_bass_kernel_spmd(nc, maps, list(range(NCORES)), **run_kwargs)


def kernel(**inputs) -> np.ndarray:
    res = run_on_cores(inputs)
    return np.concatenate(
        [res.results[i]["y"] for i in range(NCORES)], axis=0
    ).astype(np.float32)


if __name__ == "__main__":
    rng = np.random.default_rng(0)
    x = rng.standard_normal((N, F), dtype=np.float32)
    adj = rng.integers(0, 2, size=(N, N)).astype(bool)
    W = rng.standard_normal((F, F), dtype=np.float32) * 0.088
    a = rng.standard_normal((H, 2 * D), dtype=np.float32) * 0.17
    y = kernel(
        x=x, adjacency=adj, W=W, a=a,
        ln_gamma=np.ones(F, np.float32), ln_beta=np.zeros(F, np.float32),
    )
    print(y.shape, y.dtype)
